# revision 1
# baseline (speedup 1.0000x reference)
"""Multi-head cross-attention (b=2, n=m=2048, dim=1024, 16 heads) on 8 trn2 cores.

Sharding: core = be*4 + g  (be = batch element, g = head group of 4 heads).
Each core computes, for its batch element and its 4 heads:
    Q^T = (wq_g @ x1^T), K^T = (wk_g @ x2^T), V = x2 @ wv_g^T
    S^T = K^T_h-slices.T @ Q^T_h  (per head), P = exp(S * scale)  (no max
    subtraction needed, logits are ~N(0,1)), O^T = [V | 1].T @ P  (the ones
    column yields the softmax denominator for free), normalize via reciprocal
    + K=1 broadcast matmul, then y_partial = O @ wo_g^T.
Host sums the 4 head-group partials per batch element and adds the bias.

All matmuls run in float32r (TF32-like, full PE rate for free dims >= 256,
~1.5e-4 relative error). Host pre-transposes inputs so the device layout is
transpose-free. exp runs on ACT (the bottleneck engine, ~1 elem/cycle/lane)
in 1024-wide ops; head 0's S^T+exp work for the first n-chunk is emitted
inside the K/V loop so ACT starts as early as possible.
"""

import sys

if "/opt/trn_rl_repo" not in sys.path:
    sys.path.insert(0, "/opt/trn_rl_repo")

import numpy as np

import concourse.tile as tile
from concourse import bacc, mybir
from concourse import bass_utils

P = 128
NTOK = 2048            # n = m = token count per batch element
DIM = 1024
HPC = 4                # heads per core
DH = 64                # head dim
HD = HPC * DH          # 256 = per-core projection width
ECH = DIM // P         # 8 contraction chunks
NCH = NTOK // 512      # 4 n-chunks of 512
MT = NTOK // P         # 16 m-tiles of 128
SCALE = DH ** -0.5
F32 = mybir.dt.float32
F32R = mybir.dt.float32r

_CACHE: dict = {}


def _build(trace_sim: bool = False, repeat: int = 1):
    EXP = mybir.ActivationFunctionType.Exp
    nc = bacc.Bacc("TRN2", target_bir_lowering=False, debug=False, num_devices=8)
    x1T = nc.dram_tensor("x1t", [DIM, NTOK], F32R, kind="ExternalInput").ap()
    x2T = nc.dram_tensor("x2t", [DIM, NTOK], F32R, kind="ExternalInput").ap()
    wqT = nc.dram_tensor("wqt", [DIM, HD], F32R, kind="ExternalInput").ap()
    wkT = nc.dram_tensor("wkt", [DIM, HD], F32R, kind="ExternalInput").ap()
    wvT = nc.dram_tensor("wvt", [DIM, HD], F32R, kind="ExternalInput").ap()
    woT = nc.dram_tensor("wot", [HD, DIM], F32R, kind="ExternalInput").ap()
    y = nc.dram_tensor("y", [NTOK, DIM], F32, kind="ExternalOutput").ap()

    x1T_s = x1T.rearrange("(po pi) n -> pi po n", pi=P)      # [128, 8, 2048]
    x2T_s = x2T.rearrange("(po pi) n -> pi po n", pi=P)
    wqT_r = wqT.rearrange("(po pi) m -> pi po m", pi=P)      # [128, 8, 256]
    wkT_r = wkT.rearrange("(po pi) m -> pi po m", pi=P)
    wvT_r = wvT.rearrange("(po pi) m -> pi po m", pi=P)
    woT_r = woT.rearrange("(po pi) e -> pi po e", pi=P)      # [128, 2, 1024]

    with tile.TileContext(nc, trace_sim=trace_sim) as tc:
      for _rep in range(repeat):
        with (
            tc.tile_pool(name="persist", bufs=1) as persist,
            tc.tile_pool(name="ps1", bufs=4, space="PSUM") as ps1,   # [128,512]
            tc.tile_pool(name="psS", bufs=2, space="PSUM") as psSp,  # [128,1024]
            tc.tile_pool(name="xq", bufs=2) as xqpool,
        ):
            wq_sb = persist.tile([P, ECH, HD], F32R, tag="wq")
            wo_sb = persist.tile([P, 2, DIM], F32R, tag="wo")
            onesf = persist.tile([P, 64], F32, tag="onesf")
            nc.vector.memset(onesf[:], 1.0)
            QT_sb = persist.tile([P, 2, NTOK], F32R, tag="QT")
            O_sb = persist.tile([P, 2, NTOK], F32R, tag="O")
            KT_sb = persist.tile([P, 2, NTOK], F32R, tag="KT")
            V_sb = persist.tile([P, MT, HPC, 65], F32R, tag="V")
            nc.vector.tensor_copy(
                V_sb[:, :, :, 64:65],
                onesf[:].rearrange("p (a b c) -> p a b c", a=MT, b=HPC, c=1),
            )

            def q_proj(nq):
                # Q^T projection for one n-chunk (256-wide x sub-chunks)
                for half in range(2):
                    cs = slice(nq * 512 + half * 256, nq * 512 + half * 256 + 256)
                    xq = xqpool.tile([P, ECH, 256], F32R, tag="xq")
                    for ec in range(ECH):
                        nc.sync.dma_start(xq[:, ec], x1T_s[:, ec, cs])
                    for pg in range(2):
                        psq = ps1.tile(
                            [P, 512], F32, tag="b1", name=f"psq{nq}{half}{pg}"
                        )
                        for ec in range(ECH):
                            nc.tensor.matmul(
                                psq[:, 0:256],
                                wq_sb[:, ec, pg * P:(pg + 1) * P],
                                xq[:, ec, :],
                                start=(ec == 0),
                                stop=(ec == ECH - 1),
                            )
                        nc.vector.tensor_copy(QT_sb[:, pg, cs], psq[:, 0:256])

            def s_exp_pair(nq, h, mtp, expS):
                # one [128,1024] psS pair: S^T for m-tiles (2*mtp, 2*mtp+1)
                pg, off = h // 2, 64 * (h % 2)
                ns = slice(nq * 512, (nq + 1) * 512)
                psS = psSp.tile([P, 1024], F32, tag="psS", name=f"psS{nq}{h}{mtp}")
                for sub in range(2):
                    mt = 2 * mtp + sub
                    nc.tensor.matmul(
                        psS[:, sub * 512:(sub + 1) * 512],
                        KT_sb[off:off + 64, pg, mt * P:(mt + 1) * P],
                        QT_sb[off:off + 64, pg, ns],
                        start=True,
                        stop=True,
                    )
                nc.scalar.activation(
                    expS[:, 2 * mtp:2 * mtp + 2, :].rearrange("p a b -> p (a b)"),
                    psS[:],
                    EXP,
                    scale=SCALE,
                )

            with (
                tc.tile_pool(name="wkv", bufs=1) as wkvpool,
                tc.tile_pool(name="xk", bufs=2) as xkpool,
            ):
                # weights for K first (needed earliest), per-chunk DMAs
                wk_sb = wkvpool.tile([P, ECH, HD], F32R, tag="wk")
                for ec in range(ECH):
                    nc.sync.dma_start(wk_sb[:, ec], wkT_r[:, ec])
                wv_sb = wkvpool.tile([P, ECH, HD], F32R, tag="wv")

                # ---- single x2 pass: K^T projection + V projection; h0's
                # S^T+exp for the first n-chunk is emitted as K tiles land so
                # the ACT engine (bottleneck) starts early ----
                for nq in range(NCH):
                    ns = slice(nq * 512, (nq + 1) * 512)
                    xk = xkpool.tile([P, ECH, 512], F32R, tag="xk")
                    for ec in range(ECH):
                        nc.sync.dma_start(xk[:, ec], x2T_s[:, ec, ns])
                    if nq == 0:
                        for ec in range(ECH):
                            nc.sync.dma_start(wv_sb[:, ec], wvT_r[:, ec])
                    for pg in range(2):
                        psq = ps1.tile([P, 512], F32, tag="b1", name=f"psk{nq}{pg}")
                        for ec in range(ECH):
                            nc.tensor.matmul(
                                psq[:],
                                wk_sb[:, ec, pg * P:(pg + 1) * P],
                                xk[:, ec, :],
                                start=(ec == 0),
                                stop=(ec == ECH - 1),
                            )
                        nc.vector.tensor_copy(KT_sb[:, pg, ns], psq[:])
                    # V for the 4 m-tiles covered by this x2 chunk
                    for sub in range(4):
                        mt = 4 * nq + sub
                        pv = ps1.tile([P, 512], F32, tag="b1", name=f"psv{mt}")
                        for ec in range(ECH):
                            nc.tensor.matmul(
                                pv[:, 0:256],
                                xk[:, ec, sub * P:(sub + 1) * P],
                                wv_sb[:, ec, :],
                                start=(ec == 0),
                                stop=(ec == ECH - 1),
                            )
                        nc.vector.tensor_copy(
                            V_sb[:, mt, :, 0:64],
                            pv[:, 0:256].rearrange("p (h d) -> p h d", d=64),
                        )
                    if nq == 0:
                        nc.sync.dma_start(wq_sb[:], wqT_r)
                        q_proj(0)
                    if nq == NCH - 1:
                        nc.sync.dma_start(wo_sb[:], woT_r)

            # ---- per n-chunk: Q^T projection, attention, out-projection ----
            with (
                tc.tile_pool(name="exps", bufs=2) as expool,
                tc.tile_pool(name="rec", bufs=1) as recpool,
                tc.tile_pool(name="bcp", bufs=1) as bcpool,
                tc.tile_pool(name="otmp", bufs=1) as tmppool,
                tc.tile_pool(name="ysb", bufs=2) as ypool,
            ):
                for nq in range(NCH):
                    ns = slice(nq * 512, (nq + 1) * 512)
                    if nq + 1 < NCH:
                        q_proj(nq + 1)
                    for h in range(HPC):
                        pg, off = h // 2, 64 * (h % 2)
                        expS = expool.tile(
                            [P, MT, 512], F32R, tag="expS", name=f"expS{nq}{h}"
                        )
                        for mtp in range(MT // 2):
                            s_exp_pair(nq, h, mtp, expS)
                        psO = ps1.tile([P, 512], F32, tag="b1", name=f"psO{nq}{h}")
                        for mt in range(MT):
                            nc.tensor.matmul(
                                psO[0:65, :],
                                V_sb[:, mt, h, :],
                                expS[:, mt, :],
                                start=(mt == 0),
                                stop=(mt == MT - 1),
                            )
                        rec = recpool.tile([P, 512], F32R, tag="rec")
                        with nc.allow_low_precision(
                            reason="fp32r rounding of softmax denom reciprocal"
                        ):
                            nc.vector.reciprocal(rec[64:65, :], psO[64:65, :])
                        nc.sync.dma_start(rec[0:1, :], rec[64:65, :])
                        bc = bcpool.tile([64, 512], F32R, tag="bc")
                        nc.gpsimd.partition_broadcast(bc[:], rec[0:1, :])
                        if off == 0:
                            nc.vector.tensor_mul(
                                O_sb[0:64, pg, ns], psO[0:64, :], bc[:]
                            )
                        else:
                            tmp = tmppool.tile([64, 512], F32R, tag="otmp")
                            nc.vector.tensor_mul(tmp[:], psO[0:64, :], bc[:])
                            nc.sync.dma_start(O_sb[64:128, pg, ns], tmp[:])
                    # out-projection deferred one chunk: fills PE gaps while
                    # ACT works on the next chunk's exp
                    if nq > 0:
                        out_proj(nc, nq - 1, ps1, ypool, wo_sb, O_sb, y)
                out_proj(nc, NCH - 1, ps1, ypool, wo_sb, O_sb, y)
    nc.compile()
    return nc


def out_proj(nc, nq, ps1, ypool, wo_sb, O_sb, y):
    for nt in range(4 * nq, 4 * nq + 4):
        y_sb = ypool.tile([P, DIM], F32, tag="y", name=f"ysb{nt}")
        for eo in range(2):
            psY = ps1.tile([P, 512], F32, tag="b1", name=f"psY{nt}{eo}")
            for hd in range(2):
                nc.tensor.matmul(
                    psY[:],
                    O_sb[:, hd, nt * P:(nt + 1) * P],
                    wo_sb[:, hd, eo * 512:(eo + 1) * 512],
                    start=(hd == 0),
                    stop=(hd == 1),
                )
            nc.vector.tensor_copy(y_sb[:, eo * 512:(eo + 1) * 512], psY[:])
        nc.gpsimd.dma_start(y[nt * P:(nt + 1) * P, :], y_sb[:])


def get_nc(trace_sim: bool = False, repeat: int = 1):
    key = ("nc", trace_sim, repeat)
    if key not in _CACHE:
        _CACHE[key] = _build(trace_sim, repeat)
    return _CACHE[key]


def make_in_maps(x1, x2, wq, wk, wv, wo):
    x1 = np.asarray(x1, dtype=np.float32)
    x2 = np.asarray(x2, dtype=np.float32)
    wq = np.asarray(wq, dtype=np.float32)
    wk = np.asarray(wk, dtype=np.float32)
    wv = np.asarray(wv, dtype=np.float32)
    wo = np.asarray(wo, dtype=np.float32)
    in_maps = []
    for core in range(8):
        be, g = core // 4, core % 4
        sl = slice(HD * g, HD * (g + 1))
        in_maps.append({
            "x1t": np.ascontiguousarray(x1[be].T),
            "x2t": np.ascontiguousarray(x2[be].T),
            "wqt": np.ascontiguousarray(wq[sl, :].T),
            "wkt": np.ascontiguousarray(wk[sl, :].T),
            "wvt": np.ascontiguousarray(wv[sl, :].T),
            "wot": np.ascontiguousarray(wo[:, sl].T),
        })
    return in_maps


def assemble(results, bo):
    bo = np.asarray(bo, dtype=np.float32)
    out = np.empty((2, NTOK, DIM), np.float32)
    for be in range(2):
        acc = results[be * 4]["y"].copy()
        for g in range(1, 4):
            acc += results[be * 4 + g]["y"]
        out[be] = acc + bo
    return out


def kernel(x1, x2, wq, wk, wv, wo, bo):
    nc = get_nc()
    in_maps = make_in_maps(x1, x2, wq, wk, wv, wo)
    last_err = None
    for attempt in range(3):
        try:
            res = bass_utils.run_bass_kernel_spmd(
                nc, in_maps, core_ids=list(range(8))
            )
            return assemble(res.results, bo)
        except Exception as e:  # transient NRT_EXEC_UNIT_UNRECOVERABLE etc.
            last_err = e
            import time as _time
            _time.sleep(5 * (attempt + 1))
    raise last_err



# revision 3
# speedup vs baseline: 1.3743x; 1.3743x over previous
"""Multi-head cross-attention (b=2, n=m=2048, dim=1024, 16 heads) on 8 trn2 cores.

Sharding: core = be*4 + g  (be = batch element, g = head group of 4 heads).
Each core computes, for its batch element and its 4 heads:
    Q^T = (wq_g @ x1^T), K^T = (wk_g @ x2^T), V = x2 @ wv_g^T
    S^T = K^T.T @ Q^T  (per head), P = exp(S * scale), O^T = [V | 1].T @ P
    (ones column = softmax denominator), normalize via fast reciprocal +
    partition broadcast, y_partial = O @ wo_g^T.
Host sums the 4 head-group partials per batch element and adds the bias.

Key HW facts this version exploits (measured on this axon/trn2 setup):
  - matmul with contraction dim 64 runs ~2.3x slower per row than C=128
    (437 vs 171 ns per 512-row bf16 matmul). The per-head S^T matmul
    (contraction = head_dim = 64) therefore uses a PACKED stationary
    (two heads' K^T stacked on 128 partitions) against a zero-padded
    moving Q (per-head Q^T occupies its 64 rows, the other 64 rows are
    zero), which computes the same S^T at full C=128 rate.
  - bf16 matmuls are ~1.27x faster than fp32r (171 vs 217 ns / 512 rows)
    and halve SBUF + DMA traffic. fp32 psum accumulation throughout; the
    rel-err budget (2e-2) has plenty of room.
  - ACT (exp) is the second pole: 128 exps x 1024-wide from psum ~= 147us.
  - nc.vector.reciprocal is ~6 cycles/elem; reciprocal_approx_fast is 1.
All inputs are pre-transposed AND pre-converted to bf16 on the host; the
y partial is returned as bf16 (host accumulates in fp32).
"""

import sys

if "/opt/trn_rl_repo" not in sys.path:
    sys.path.insert(0, "/opt/trn_rl_repo")

import numpy as np

import concourse.tile as tile
from concourse import bacc, mybir
from concourse import bass_utils

P = 128
NTOK = 2048            # n = m = token count per batch element
DIM = 1024
HPC = 4                # heads per core
DH = 64                # head dim
HD = HPC * DH          # 256 = per-core projection width
ECH = DIM // P         # 8 contraction chunks
NCH = NTOK // 512      # 4 n-chunks of 512
MT = NTOK // P         # 16 m-tiles of 128
SCALE = DH ** -0.5
F32 = mybir.dt.float32
BF16 = mybir.dt.bfloat16

_CACHE: dict = {}


def _build(trace_sim: bool = False, repeat: int = 1):
    EXP = mybir.ActivationFunctionType.Exp
    nc = bacc.Bacc("TRN2", target_bir_lowering=False, debug=False, num_devices=8)
    x1T = nc.dram_tensor("x1t", [DIM, NTOK], BF16, kind="ExternalInput").ap()
    x2T = nc.dram_tensor("x2t", [DIM, NTOK], BF16, kind="ExternalInput").ap()
    wqT = nc.dram_tensor("wqt", [DIM, HD], BF16, kind="ExternalInput").ap()
    wkT = nc.dram_tensor("wkt", [DIM, HD], BF16, kind="ExternalInput").ap()
    wvT = nc.dram_tensor("wvt", [DIM, HD], BF16, kind="ExternalInput").ap()
    woT = nc.dram_tensor("wot", [HD, DIM], BF16, kind="ExternalInput").ap()
    y = nc.dram_tensor("y", [NTOK, DIM], BF16, kind="ExternalOutput").ap()

    x1T_s = x1T.rearrange("(po pi) n -> pi po n", pi=P)      # [128, 8, 2048]
    x2T_s = x2T.rearrange("(po pi) n -> pi po n", pi=P)
    wqT_r = wqT.rearrange("(po pi) m -> pi po m", pi=P)      # [128, 8, 256]
    wkT_r = wkT.rearrange("(po pi) m -> pi po m", pi=P)
    wvT_r = wvT.rearrange("(po pi) m -> pi po m", pi=P)
    woT_r = woT.rearrange("(po pi) e -> pi po e", pi=P)      # [128, 2, 1024]

    with tile.TileContext(nc, trace_sim=trace_sim) as tc:
      for _rep in range(repeat):
        with (
            tc.tile_pool(name="persist", bufs=1) as persist,
            tc.tile_pool(name="ps1", bufs=4, space="PSUM") as ps1,   # [128,512]
            tc.tile_pool(name="psS", bufs=2, space="PSUM") as psSp,  # [128,1024]
            tc.tile_pool(name="xq", bufs=2) as xqpool,
            tc.tile_pool(name="exps", bufs=2) as expool,
            tc.tile_pool(name="rec", bufs=2) as recpool,
            tc.tile_pool(name="bcp", bufs=2) as bcpool,
            tc.tile_pool(name="otmp", bufs=2) as tmppool,
            tc.tile_pool(name="ysb", bufs=2) as ypool,
        ):
            wq_sb = persist.tile([P, ECH, HD], BF16, tag="wq")
            wo_sb = persist.tile([P, 2, DIM], BF16, tag="wo")
            onesf = persist.tile([P, 64], BF16, tag="onesf")
            nc.vector.memset(onesf[:], 1.0)
            # warm the ACT exp table during initial DMAs
            dum = persist.tile([P, 8], F32, tag="dum")
            nc.vector.memset(dum[:], 0.0)
            nc.scalar.activation(dum[:], dum[:], EXP)
            # zero-padded per-head Q^T: head h occupies rows 64*(h%2)..+64 of
            # QTz[:, h, :]; the other 64 rows stay zero so the S^T matmul can
            # run with the full packed K^T stationary (C=128).
            QTz = persist.tile([P, HPC, NTOK], BF16, tag="QTz")
            nc.vector.memset(QTz[:], 0.0)
            O_sb = persist.tile([P, 2, NTOK], BF16, tag="O")
            KT_sb = persist.tile([P, 2, NTOK], BF16, tag="KT")
            V_sb = persist.tile([P, MT, HPC, 65], BF16, tag="V")
            nc.vector.tensor_copy(
                V_sb[:, :, :, 64:65],
                onesf[:].rearrange("p (a b c) -> p a b c", a=MT, b=HPC, c=1),
            )

            def q_proj(nq):
                # Q^T projection for one n-chunk (512 wide)
                ns = slice(nq * 512, (nq + 1) * 512)
                xq = xqpool.tile([P, ECH, 512], BF16, tag="xq")
                for ec in range(ECH):
                    nc.sync.dma_start(xq[:, ec], x1T_s[:, ec, ns])
                for pg in range(2):
                    psq = ps1.tile([P, 512], F32, tag="b1", name=f"psq{nq}{pg}")
                    for ec in range(ECH):
                        nc.tensor.matmul(
                            psq[:],
                            wq_sb[:, ec, pg * P:(pg + 1) * P],
                            xq[:, ec, :],
                            start=(ec == 0),
                            stop=(ec == ECH - 1),
                        )
                    # rows 0:64 = head (pg,0), rows 64:128 = head (pg,1);
                    # both land lane-aligned in their QTz slots.
                    nc.vector.tensor_copy(QTz[0:64, 2 * pg, ns], psq[0:64, :])
                    nc.vector.tensor_copy(
                        QTz[64:128, 2 * pg + 1, ns], psq[64:128, :]
                    )

            def s_exp_pair(nq, h, mtp, expS):
                # one [128,1024] psS pair: S^T for m-tiles (2*mtp, 2*mtp+1)
                pg = h // 2
                ns = slice(nq * 512, (nq + 1) * 512)
                psS = psSp.tile([P, 1024], F32, tag="psS", name=f"psS{nq}{h}{mtp}")
                for sub in range(2):
                    mt = 2 * mtp + sub
                    nc.tensor.matmul(
                        psS[:, sub * 512:(sub + 1) * 512],
                        KT_sb[:, pg, mt * P:(mt + 1) * P],
                        QTz[:, h, ns],
                        start=True,
                        stop=True,
                    )
                nc.scalar.activation(
                    expS[:, mtp * 1024:(mtp + 1) * 1024],
                    psS[:],
                    EXP,
                    scale=SCALE,
                )

            def finish_group(nq, h, expS):
                # O^T accumulation + softmax normalization for one (nq, h)
                pg, pos = h // 2, h % 2
                ns = slice(nq * 512, (nq + 1) * 512)
                psO = ps1.tile([P, 512], F32, tag="b1", name=f"psO{nq}{h}")
                for mt in range(MT):
                    nc.tensor.matmul(
                        psO[0:65, :],
                        V_sb[:, mt, h, :],
                        expS[:, mt * 512:(mt + 1) * 512],
                        start=(mt == 0),
                        stop=(mt == MT - 1),
                    )
                rec = recpool.tile([P, 512], F32, tag="rec")
                with nc.allow_low_precision(
                    reason="reciprocal of softmax denom"
                ):
                    nc.vector.reciprocal(rec[64:65, :], psO[64:65, :])
                nc.sync.dma_start(rec[0:1, :], rec[64:65, :])
                bc = bcpool.tile([64, 512], F32, tag="bc")
                nc.gpsimd.partition_broadcast(bc[:], rec[0:1, :])
                if pos == 0:
                    nc.vector.tensor_mul(O_sb[0:64, pg, ns], psO[0:64, :], bc[:])
                else:
                    tmp = tmppool.tile([64, 512], BF16, tag="otmp")
                    nc.vector.tensor_mul(tmp[:], psO[0:64, :], bc[:])
                    nc.sync.dma_start(O_sb[64:128, pg, ns], tmp[:])

            def out_proj(nq):
                for nt in range(4 * nq, 4 * nq + 4):
                    y_sb = ypool.tile([P, DIM], BF16, tag="y", name=f"ysb{nt}")
                    for eo in range(2):
                        psY = ps1.tile([P, 512], F32, tag="b1", name=f"psY{nt}{eo}")
                        for pg in range(2):
                            nc.tensor.matmul(
                                psY[:],
                                O_sb[:, pg, nt * P:(nt + 1) * P],
                                wo_sb[:, pg, eo * 512:(eo + 1) * 512],
                                start=(pg == 0),
                                stop=(pg == 1),
                            )
                        nc.vector.tensor_copy(y_sb[:, eo * 512:(eo + 1) * 512], psY[:])
                    nc.gpsimd.dma_start(y[nt * P:(nt + 1) * P, :], y_sb[:])

            expS_tiles = {}

            def get_expS(idx):
                if idx not in expS_tiles:
                    expS_tiles[idx] = expool.tile(
                        [P, MT * 512], BF16, tag="expS", name=f"expS{idx}"
                    )
                return expS_tiles[idx]

            with (
                tc.tile_pool(name="wkv", bufs=1) as wkvpool,
                tc.tile_pool(name="xk", bufs=2) as xkpool,
            ):
                # weights for K first (needed earliest), per-chunk DMAs
                wk_sb = wkvpool.tile([P, ECH, HD], BF16, tag="wk")
                for ec in range(ECH):
                    nc.sync.dma_start(wk_sb[:, ec], wkT_r[:, ec])
                wv_sb = wkvpool.tile([P, ECH, HD], BF16, tag="wv")

                # ---- single x2 pass: K^T + V projections; S^T+exp for the
                # first n-chunk's heads 0,1 is emitted as K tiles land so the
                # ACT engine (a main pole) starts as early as possible ----
                for nq in range(NCH):
                    ns = slice(nq * 512, (nq + 1) * 512)
                    xk = xkpool.tile([P, ECH, 512], BF16, tag="xk")
                    for ec in range(ECH):
                        nc.sync.dma_start(xk[:, ec], x2T_s[:, ec, ns])
                    if nq == 0:
                        for ec in range(ECH):
                            nc.sync.dma_start(wv_sb[:, ec], wvT_r[:, ec])
                    for pg in range(2):
                        psq = ps1.tile([P, 512], F32, tag="b1", name=f"psk{nq}{pg}")
                        for ec in range(ECH):
                            nc.tensor.matmul(
                                psq[:],
                                wk_sb[:, ec, pg * P:(pg + 1) * P],
                                xk[:, ec, :],
                                start=(ec == 0),
                                stop=(ec == ECH - 1),
                            )
                        nc.vector.tensor_copy(KT_sb[:, pg, ns], psq[:])
                    # V for the 4 m-tiles covered by this x2 chunk
                    for sub in range(4):
                        mt = 4 * nq + sub
                        pv = ps1.tile([P, 512], F32, tag="b1", name=f"psv{mt}")
                        for ec in range(ECH):
                            nc.tensor.matmul(
                                pv[:, 0:256],
                                xk[:, ec, sub * P:(sub + 1) * P],
                                wv_sb[:, ec, :],
                                start=(ec == 0),
                                stop=(ec == ECH - 1),
                            )
                        nc.vector.tensor_copy(
                            V_sb[:, mt, :, 0:64],
                            pv[:, 0:256].rearrange("p (h d) -> p h d", d=64),
                        )
                    if nq == 0:
                        nc.sync.dma_start(wq_sb[:], wqT_r)
                        q_proj(0)
                    if nq == NCH - 1:
                        nc.sync.dma_start(wo_sb[:], woT_r)
                    # warmup attention: S^T+exp for (nq0=0, h=0/1) over the
                    # m-tiles this K chunk just produced
                    for h in range(2):
                        for mtp in (2 * nq, 2 * nq + 1):
                            s_exp_pair(0, h, mtp, get_expS(h))

            # ---- main loop over (nq, h) groups, O deferred one group so the
            # next group's S/exp stream keeps ACT busy during O+normalize ----
            groups = [(nq, h) for nq in range(NCH) for h in range(HPC)]
            for idx, (nq, h) in enumerate(groups):
                if idx >= 2:  # groups 0,1 were emitted in the warmup
                    expS = get_expS(idx)
                    for mtp in range(MT // 2):
                        s_exp_pair(nq, h, mtp, expS)
                if idx >= 1:
                    pnq, ph = groups[idx - 1]
                    finish_group(pnq, ph, get_expS(idx - 1))
                if h == 2 and nq + 1 < NCH:
                    q_proj(nq + 1)
                if h == 3 and nq > 0:
                    out_proj(nq - 1)
            finish_group(*groups[-1], get_expS(len(groups) - 1))
            out_proj(NCH - 1)
    nc.compile()
    return nc


def get_nc(trace_sim: bool = False, repeat: int = 1):
    key = ("nc", trace_sim, repeat)
    if key not in _CACHE:
        _CACHE[key] = _build(trace_sim, repeat)
    return _CACHE[key]


def make_in_maps(x1, x2, wq, wk, wv, wo):
    import ml_dtypes
    bf16 = ml_dtypes.bfloat16
    x1 = np.asarray(x1, dtype=np.float32)
    x2 = np.asarray(x2, dtype=np.float32)
    wq = np.asarray(wq, dtype=np.float32)
    wk = np.asarray(wk, dtype=np.float32)
    wv = np.asarray(wv, dtype=np.float32)
    wo = np.asarray(wo, dtype=np.float32)
    in_maps = []
    for core in range(8):
        be, g = core // 4, core % 4
        sl = slice(HD * g, HD * (g + 1))
        in_maps.append({
            "x1t": np.ascontiguousarray(x1[be].T).astype(bf16),
            "x2t": np.ascontiguousarray(x2[be].T).astype(bf16),
            "wqt": np.ascontiguousarray(wq[sl, :].T).astype(bf16),
            "wkt": np.ascontiguousarray(wk[sl, :].T).astype(bf16),
            "wvt": np.ascontiguousarray(wv[sl, :].T).astype(bf16),
            "wot": np.ascontiguousarray(wo[:, sl].T).astype(bf16),
        })
    return in_maps


def assemble(results, bo):
    bo = np.asarray(bo, dtype=np.float32)
    out = np.empty((2, NTOK, DIM), np.float32)
    for be in range(2):
        acc = results[be * 4]["y"].astype(np.float32)
        for g in range(1, 4):
            acc += results[be * 4 + g]["y"].astype(np.float32)
        out[be] = acc + bo
    return out


def kernel(x1, x2, wq, wk, wv, wo, bo):
    nc = get_nc()
    in_maps = make_in_maps(x1, x2, wq, wk, wv, wo)
    last_err = None
    for attempt in range(3):
        try:
            res = bass_utils.run_bass_kernel_spmd(
                nc, in_maps, core_ids=list(range(8))
            )
            return assemble(res.results, bo)
        except Exception as e:  # transient NRT_EXEC_UNIT_UNRECOVERABLE etc.
            last_err = e
            import time as _time
            _time.sleep(5 * (attempt + 1))
    raise last_err


# revision 5
# speedup vs baseline: 1.6761x; 1.2196x over previous
"""Multi-head cross-attention (b=2, n=m=2048, dim=1024, 16 heads) on 8 trn2 cores.

Sharding: core = be*4 + g  (be = batch element, g = head group of 4 heads).
Each core computes, for its batch element and its 4 heads:
    Q^T = (wq_g @ x1^T), K^T = (wk_g @ x2^T), V = x2 @ wv_g^T
    S^T = K^T.T @ Q^T  (per head), P = exp(S * scale), O^T = [V | 1].T @ P
    (ones column = softmax denominator), normalize via reciprocal +
    partition broadcast, y_partial = O @ wo_g^T.
Host sums the 4 head-group partials per batch element and adds the bias.

HW facts this version exploits (measured on this axon/trn2 setup):
  - matmul with contraction dim 64 runs ~2.3x slower per row than C=128
    (437 vs 171 ns per 512-row bf16 matmul). The per-head S^T matmul
    (contraction = head_dim = 64) therefore uses the PACKED two-head K^T
    stationary (128 partitions) against a zero-padded per-head Q^T moving
    operand (the head's 64 rows hold Q^T, the other 64 rows are zero),
    computing the same S^T at full C=128 rate.
  - bf16 matmuls are ~1.27x faster than fp32r and halve SBUF/DMA traffic.
  - Engine queues execute IN ORDER, so emission order is the schedule.
    The exp stream (ACT, ~147us total) must never be starved: S^T pairs,
    the previous group's O accumulation, and out/Q-projection work are
    interleaved at ~1 slab granularity so no PE run between S-pairs
    exceeds the ~1.15us exp slab time.
"""

import sys

if "/opt/trn_rl_repo" not in sys.path:
    sys.path.insert(0, "/opt/trn_rl_repo")

import numpy as np

import concourse.tile as tile
from concourse import bacc, mybir
from concourse import bass_utils

P = 128
NTOK = 2048            # n = m = token count per batch element
DIM = 1024
HPC = 4                # heads per core
DH = 64                # head dim
HD = HPC * DH          # 256 = per-core projection width
ECH = DIM // P         # 8 contraction chunks
NCH = NTOK // 512      # 4 n-chunks of 512
MT = NTOK // P         # 16 m-tiles of 128
NG = NCH * HPC         # 16 (nq, h) groups
SCALE = DH ** -0.5
F32 = mybir.dt.float32
BF16 = mybir.dt.bfloat16

_CACHE: dict = {}


def _build(trace_sim: bool = False, repeat: int = 1):
    EXP = mybir.ActivationFunctionType.Exp
    nc = bacc.Bacc("TRN2", target_bir_lowering=False, debug=False, num_devices=8)
    x1T = nc.dram_tensor("x1t", [DIM, NTOK], BF16, kind="ExternalInput").ap()
    x2T = nc.dram_tensor("x2t", [DIM, NTOK], BF16, kind="ExternalInput").ap()
    wqT = nc.dram_tensor("wqt", [DIM, HD], BF16, kind="ExternalInput").ap()
    wkT = nc.dram_tensor("wkt", [DIM, HD], BF16, kind="ExternalInput").ap()
    wvT = nc.dram_tensor("wvt", [DIM, HD], BF16, kind="ExternalInput").ap()
    woT = nc.dram_tensor("wot", [HD, DIM], BF16, kind="ExternalInput").ap()
    y = nc.dram_tensor("y", [NTOK, DIM], BF16, kind="ExternalOutput").ap()

    x1T_s = x1T.rearrange("(po pi) n -> pi po n", pi=P)      # [128, 8, 2048]
    x2T_s = x2T.rearrange("(po pi) n -> pi po n", pi=P)
    wqT_r = wqT.rearrange("(po pi) m -> pi po m", pi=P)      # [128, 8, 256]
    wkT_r = wkT.rearrange("(po pi) m -> pi po m", pi=P)
    wvT_r = wvT.rearrange("(po pi) m -> pi po m", pi=P)
    woT_r = woT.rearrange("(po pi) e -> pi po e", pi=P)      # [128, 2, 1024]

    with tile.TileContext(nc, trace_sim=trace_sim) as tc:
      for _rep in range(repeat):
        with (
            tc.tile_pool(name="persist", bufs=1) as persist,
            tc.tile_pool(name="ps1", bufs=4, space="PSUM") as ps1,   # [128,512]
            tc.tile_pool(name="psS", bufs=2, space="PSUM") as psSp,  # [128,1024]
            tc.tile_pool(name="xq", bufs=2) as xqpool,
            tc.tile_pool(name="slab", bufs=24) as slabpool,
            tc.tile_pool(name="rec", bufs=2) as recpool,
            tc.tile_pool(name="bcp", bufs=2) as bcpool,
            tc.tile_pool(name="otmp", bufs=2) as tmppool,
            tc.tile_pool(name="ysb", bufs=2) as ypool,
        ):
            wq_sb = persist.tile([P, ECH, HD], BF16, tag="wq")
            wo_sb = persist.tile([P, 2, DIM], BF16, tag="wo")
            onesf = persist.tile([P, 64], BF16, tag="onesf")
            nc.vector.memset(onesf[:], 1.0)
            # warm the ACT exp table during initial DMAs
            dum = persist.tile([P, 8], F32, tag="dum")
            nc.vector.memset(dum[:], 0.0)
            nc.scalar.activation(dum[:], dum[:], EXP)
            # zero-padded per-head Q^T: head h occupies rows 64*(h%2)..+64 of
            # QTz[:, h, :]; the other 64 rows stay zero so the S^T matmul can
            # use the packed two-head K^T stationary at C=128.
            QTz = persist.tile([P, HPC, NTOK], BF16, tag="QTz")
            nc.vector.memset(QTz[:], 0.0)
            O_sb = persist.tile([P, 2, NTOK], BF16, tag="O")
            KT_sb = persist.tile([P, 2, NTOK], BF16, tag="KT")
            V_sb = persist.tile([P, MT, HPC, 65], BF16, tag="V")
            nc.vector.tensor_copy(
                V_sb[:, :, :, 64:65],
                onesf[:].rearrange("p (a b c) -> p a b c", a=MT, b=HPC, c=1),
            )

            slabs = {}

            def s_exp_pair(g, mtp):
                # S^T + exp for m-tiles (2*mtp, 2*mtp+1) of group g=(nq,h)
                nq, h = g // HPC, g % HPC
                pg = h // 2
                ns = slice(nq * 512, (nq + 1) * 512)
                psS = psSp.tile([P, 1024], F32, tag="psS", name=f"psS{g}_{mtp}")
                for sub in range(2):
                    mt = 2 * mtp + sub
                    nc.tensor.matmul(
                        psS[:, sub * 512:(sub + 1) * 512],
                        KT_sb[:, pg, mt * P:(mt + 1) * P],
                        QTz[:, h, ns],
                        start=True,
                        stop=True,
                    )
                es = slabpool.tile([P, 1024], BF16, tag="es", name=f"es{g}_{mtp}")
                slabs[(g, mtp)] = es
                nc.scalar.activation(es[:], psS[:], EXP, scale=SCALE)

            psO_t = {}

            def o_pair(g, mtp):
                # two O^T accumulation steps for group g (m-tiles 2mtp,2mtp+1)
                h = g % HPC
                if mtp == 0:
                    psO_t[g] = ps1.tile([P, 512], F32, tag="b1", name=f"psO{g}")
                psO = psO_t[g]
                es = slabs[(g, mtp)]
                for sub in range(2):
                    mt = 2 * mtp + sub
                    nc.tensor.matmul(
                        psO[0:65, :],
                        V_sb[:, mt, h, :],
                        es[:, sub * 512:(sub + 1) * 512],
                        start=(mt == 0),
                        stop=(mt == MT - 1),
                    )

            def normalize(g):
                nq, h = g // HPC, g % HPC
                pg, pos = h // 2, h % 2
                ns = slice(nq * 512, (nq + 1) * 512)
                psO = psO_t.pop(g)
                rec = recpool.tile([P, 512], F32, tag="rec")
                with nc.allow_low_precision(reason="softmax denom reciprocal"):
                    nc.vector.reciprocal(rec[64:65, :], psO[64:65, :])
                nc.sync.dma_start(rec[0:1, :], rec[64:65, :])
                bc = bcpool.tile([64, 512], F32, tag="bc")
                nc.gpsimd.partition_broadcast(bc[:], rec[0:1, :])
                if pos == 0:
                    nc.vector.tensor_mul(O_sb[0:64, pg, ns], psO[0:64, :], bc[:])
                else:
                    tmp = tmppool.tile([64, 512], BF16, tag="otmp")
                    nc.vector.tensor_mul(tmp[:], psO[0:64, :], bc[:])
                    nc.sync.dma_start(O_sb[64:128, pg, ns], tmp[:])

            # --- small PE work pieces, popped between slabs -----------------
            extra_q = []

            def qproj_pieces(nq):
                # Q^T projection for n-chunk nq, split into ~2-matmul pieces
                ns = slice(nq * 512, (nq + 1) * 512)
                state = {}

                def dma_piece():
                    xq = xqpool.tile([P, ECH, 512], BF16, tag="xq")
                    state["xq"] = xq
                    for ec in range(ECH):
                        nc.sync.dma_start(xq[:, ec], x1T_s[:, ec, ns])

                def mm_piece(pg, e0):
                    def run():
                        if e0 == 0:
                            state[pg] = ps1.tile(
                                [P, 512], F32, tag="b1", name=f"psq{nq}{pg}"
                            )
                        psq = state[pg]
                        for ec in (e0, e0 + 1):
                            nc.tensor.matmul(
                                psq[:],
                                wq_sb[:, ec, pg * P:(pg + 1) * P],
                                state["xq"][:, ec, :],
                                start=(ec == 0),
                                stop=(ec == ECH - 1),
                            )
                        if e0 == ECH - 2:
                            nc.vector.tensor_copy(
                                QTz[0:64, 2 * pg, ns], psq[0:64, :]
                            )
                            nc.vector.tensor_copy(
                                QTz[64:128, 2 * pg + 1, ns], psq[64:128, :]
                            )
                    return run

                yield dma_piece
                for pg in range(2):
                    for e0 in range(0, ECH, 2):
                        yield mm_piece(pg, e0)

            def outproj_pieces(nq):
                state = {}

                def mm_piece(nt, eo):
                    def run():
                        if eo == 0:
                            state[nt] = ypool.tile(
                                [P, DIM], BF16, tag="y", name=f"ysb{nt}"
                            )
                        psY = ps1.tile([P, 512], F32, tag="b1", name=f"psY{nt}{eo}")
                        for pg in range(2):
                            nc.tensor.matmul(
                                psY[:],
                                O_sb[:, pg, nt * P:(nt + 1) * P],
                                wo_sb[:, pg, eo * 512:(eo + 1) * 512],
                                start=(pg == 0),
                                stop=(pg == 1),
                            )
                        nc.vector.tensor_copy(
                            state[nt][:, eo * 512:(eo + 1) * 512], psY[:]
                        )
                        if eo == 1:
                            nc.gpsimd.dma_start(
                                y[nt * P:(nt + 1) * P, :], state[nt][:]
                            )
                    return run

                for nt in range(4 * nq, 4 * nq + 4):
                    for eo in range(2):
                        yield mm_piece(nt, eo)

            def pop_extra(k=1):
                for _ in range(k):
                    if extra_q:
                        extra_q.pop(0)()

            # ---- warmup: single x2 pass: K^T + V projections, with groups
            # 0 and 1 (nq=0, h=0/1) S^T+exp emitted as K chunks land ----
            with (
                tc.tile_pool(name="wkv", bufs=1) as wkvpool,
                tc.tile_pool(name="xk", bufs=2) as xkpool,
            ):
                wk_sb = wkvpool.tile([P, ECH, HD], BF16, tag="wk")
                for ec in range(ECH):
                    nc.sync.dma_start(wk_sb[:, ec], wkT_r[:, ec])
                wv_sb = wkvpool.tile([P, ECH, HD], BF16, tag="wv")

                for nq in range(NCH):
                    ns = slice(nq * 512, (nq + 1) * 512)
                    xk = xkpool.tile([P, ECH, 512], BF16, tag="xk")
                    for ec in range(ECH):
                        nc.sync.dma_start(xk[:, ec], x2T_s[:, ec, ns])
                    if nq == 0:
                        for ec in range(ECH):
                            nc.sync.dma_start(wv_sb[:, ec], wvT_r[:, ec])
                    for pg in range(2):
                        psq = ps1.tile([P, 512], F32, tag="b1", name=f"psk{nq}{pg}")
                        for ec in range(ECH):
                            nc.tensor.matmul(
                                psq[:],
                                wk_sb[:, ec, pg * P:(pg + 1) * P],
                                xk[:, ec, :],
                                start=(ec == 0),
                                stop=(ec == ECH - 1),
                            )
                        nc.vector.tensor_copy(KT_sb[:, pg, ns], psq[:])
                    # V for the 4 m-tiles covered by this x2 chunk
                    for sub in range(4):
                        mt = 4 * nq + sub
                        pv = ps1.tile([P, 512], F32, tag="b1", name=f"psv{mt}")
                        for ec in range(ECH):
                            nc.tensor.matmul(
                                pv[:, 0:256],
                                xk[:, ec, sub * P:(sub + 1) * P],
                                wv_sb[:, ec, :],
                                start=(ec == 0),
                                stop=(ec == ECH - 1),
                            )
                        nc.vector.tensor_copy(
                            V_sb[:, mt, :, 0:64],
                            pv[:, 0:256].rearrange("p (h d) -> p h d", d=64),
                        )
                    if nq == 0:
                        nc.sync.dma_start(wq_sb[:], wqT_r)
                        for piece in qproj_pieces(0):
                            piece()
                    if nq == NCH - 1:
                        nc.sync.dma_start(wo_sb[:], woT_r)
                    # warmup S/exp: groups 0,1 (h=0,1 need only pg=0 K rows,
                    # both live in the packed KT chunk just produced)
                    for h in range(2):
                        for mtp in (2 * nq, 2 * nq + 1):
                            s_exp_pair(h, mtp)

            # ---- main software pipeline over groups: S(idx) slabs are
            # interleaved with O(idx-1) pairs and small projection pieces ----
            for idx in range(1, NG):
                nq, h = idx // HPC, idx % HPC
                if h == 1 and nq + 1 < NCH:
                    extra_q.extend(qproj_pieces(nq + 1))
                if h == 2 and nq >= 1:
                    extra_q.extend(outproj_pieces(nq - 1))
                for mtp in range(MT // 2):
                    if idx >= 2:
                        s_exp_pair(idx, mtp)
                    o_pair(idx - 1, mtp)
                    pop_extra(1)
                normalize(idx - 1)
            for mtp in range(MT // 2):
                o_pair(NG - 1, mtp)
                pop_extra(2)
            normalize(NG - 1)
            while extra_q:
                pop_extra(1)
            for piece in outproj_pieces(NCH - 1):
                piece()
    nc.compile()
    return nc


def get_nc(trace_sim: bool = False, repeat: int = 1):
    key = ("nc", trace_sim, repeat)
    if key not in _CACHE:
        _CACHE[key] = _build(trace_sim, repeat)
    return _CACHE[key]


def make_in_maps(x1, x2, wq, wk, wv, wo):
    import ml_dtypes
    bf16 = ml_dtypes.bfloat16
    x1 = np.asarray(x1, dtype=np.float32)
    x2 = np.asarray(x2, dtype=np.float32)
    wq = np.asarray(wq, dtype=np.float32)
    wk = np.asarray(wk, dtype=np.float32)
    wv = np.asarray(wv, dtype=np.float32)
    wo = np.asarray(wo, dtype=np.float32)
    in_maps = []
    for core in range(8):
        be, g = core // 4, core % 4
        sl = slice(HD * g, HD * (g + 1))
        in_maps.append({
            "x1t": np.ascontiguousarray(x1[be].T).astype(bf16),
            "x2t": np.ascontiguousarray(x2[be].T).astype(bf16),
            "wqt": np.ascontiguousarray(wq[sl, :].T).astype(bf16),
            "wkt": np.ascontiguousarray(wk[sl, :].T).astype(bf16),
            "wvt": np.ascontiguousarray(wv[sl, :].T).astype(bf16),
            "wot": np.ascontiguousarray(wo[:, sl].T).astype(bf16),
        })
    return in_maps


def assemble(results, bo):
    bo = np.asarray(bo, dtype=np.float32)
    out = np.empty((2, NTOK, DIM), np.float32)
    for be in range(2):
        acc = results[be * 4]["y"].astype(np.float32)
        for g in range(1, 4):
            acc += results[be * 4 + g]["y"].astype(np.float32)
        out[be] = acc + bo
    return out


def kernel(x1, x2, wq, wk, wv, wo, bo):
    nc = get_nc()
    in_maps = make_in_maps(x1, x2, wq, wk, wv, wo)
    last_err = None
    for attempt in range(3):
        try:
            res = bass_utils.run_bass_kernel_spmd(
                nc, in_maps, core_ids=list(range(8))
            )
            return assemble(res.results, bo)
        except Exception as e:  # transient NRT_EXEC_UNIT_UNRECOVERABLE etc.
            last_err = e
            import time as _time
            _time.sleep(5 * (attempt + 1))
    raise last_err
